# revision 12
# baseline (speedup 1.0000x reference)
"""Causal self-attention (B=4, T=2048, C=1024, H=16, D=64) on 8 TRN2 NeuronCores.

Sharding: data-parallel over batch (4) x tensor-parallel over heads (2 groups
of 8 heads).  Core c handles batch c//2 and heads (c%2)*8 .. (c%2)*8+8.
Each core computes its QKV projection shard, causal flash-style attention for
its 8 heads, and a partial output projection (row-parallel).  The host sums
the two partials per batch and adds b_proj.

v2 over the baseline:
  - QK matmuls for the two heads of a pair are emitted adjacently at row
    groups (0,0)/(64,0) so they execute concurrently (contraction is 64).
  - Causal-diagonal trimming: for query chunk I, the 4 diagonal kv blocks
    stream only the queries that can attend (N = 512-128r), in QK, exp, and
    AV.  Masks shrink to one [128,128] triangle per diagonal block.
  - QKV bias folded into the PSUM->SBUF copy (DVE tensor_scalar); V bias via
    a pre-broadcast [128,512] tile (DVE tensor_tensor).  No bias matmuls.
  - All weights/activations DMA'd from host-prearranged partition-major
    layouts (>=2KB contiguous per partition line); xT streamed per 512-token
    chunk so matmuls start ~4us in.
  - Softmax epilogue per query-chunk (not per pair): yT chunks land early,
    letting the output projection for token chunk tch start right after
    pair 3's chunk tch instead of after the whole pair.
  - Emission interleaves QKV/V units with pair-0 attention chunks, remaining
    QKV m-tiles with pairs 1-2, projection with pair 3 (the per-engine
    queues are static FIFO, so emission order is the schedule).
  - outT stored bf16 (halves output DMA), summed on host in f32.

All matmuls bf16 with f32 PSUM accumulation; scores exp'd on ACT (the only
exp engine); copies pinned to DVE so ACT does exp exclusively.
"""

import os
import sys

for _p in (
    "/root/.axon_site",
    "/root/.axon_site/_ro/trn_rl_repo",
    "/root/.axon_site/_ro/pypackages",
    "/opt/trn_rl_repo",
):
    if os.path.isdir(_p) and _p not in sys.path:
        sys.path.append(_p)

from contextlib import ExitStack

import ml_dtypes
import numpy as np

import concourse.tile as tile
from concourse import bacc, mybir
from concourse.bass import ds, ts
from concourse.bass_utils import run_bass_kernel_spmd

F32 = mybir.dt.float32
F32R = mybir.dt.float32r
BF16 = mybir.dt.bfloat16
EXP = mybir.ActivationFunctionType.Exp
MULT = mybir.AluOpType.mult
ADD = mybir.AluOpType.add

B, T, C, H, D = 4, 2048, 1024, 16, 64
HPC = 8            # heads per core
CS = HPC * D       # 512 sharded channels
NC_ = C // 128     # 8 channel tiles
TB = T // 128      # 16 token blocks
TCH = T // 512     # 4 query chunks
SCALE = 1.0 / np.sqrt(D)

_CACHE = {}


def _build_program():
    nc = bacc.Bacc("TRN2", target_bir_lowering=False, debug=False)

    xt_d = nc.dram_tensor("xt", [TCH, 128, NC_, 512], BF16, kind="ExternalInput")
    wqk_d = nc.dram_tensor("wqk", [8, 128, NC_, 128], BF16, kind="ExternalInput")
    bqk_d = nc.dram_tensor("bqk", [128, 8], F32, kind="ExternalInput")
    wv_d = nc.dram_tensor("wv", [128, NC_, CS], BF16, kind="ExternalInput")
    bv_d = nc.dram_tensor("bv", [128, CS], F32, kind="ExternalInput")
    wp_d = nc.dram_tensor("wp", [128, 4, C], BF16, kind="ExternalInput")
    mk_d = nc.dram_tensor("mask", [128, 128], BF16, kind="ExternalInput")
    outT_d = nc.dram_tensor("outT", [8, 128, T], BF16, kind="ExternalOutput")

    with tile.TileContext(nc) as tc, ExitStack() as ctx, \
            nc.allow_low_precision(reason="bf16 matmuls, f32r epilogue"):
        pool_c = ctx.enter_context(tc.tile_pool(name="const", bufs=1))
        pool_qk = ctx.enter_context(tc.tile_pool(name="qkT", bufs=1))
        pool_ve = ctx.enter_context(tc.tile_pool(name="vext", bufs=1))
        pool_x = ctx.enter_context(tc.tile_pool(name="xT", bufs=1))
        pool_p = ctx.enter_context(tc.tile_pool(name="P", bufs=4))
        pool_r = ctx.enter_context(tc.tile_pool(name="rec", bufs=4))
        pool_tmp = ctx.enter_context(tc.tile_pool(name="psb", bufs=5))
        pool_y = ctx.enter_context(tc.tile_pool(name="yT", bufs=1))
        pool_o = ctx.enter_context(tc.tile_pool(name="out", bufs=3))
        ps_qk = ctx.enter_context(tc.tile_pool(name="psqk", bufs=2, space="PSUM"))
        ps_y = ctx.enter_context(tc.tile_pool(name="psy", bufs=2, space="PSUM"))
        ps_sm = ctx.enter_context(tc.tile_pool(name="pssm", bufs=2, space="PSUM"))

        # v_ext[p, tb, h*65+d]; col h*65+64 holds ones (denominator trick),
        # set by a DVE memset (a DMA here would be descriptor-bound: 16B
        # descriptors that clog the HWDGE ring for ~20us).
        vext = pool_ve.tile([128, TB, HPC * 65], BF16)
        nc.vector.memset(
            vext[:].rearrange("p tb (h s) -> p tb h s", s=65)[:, :, :, 64], 1.0
        )

        # xt streams on the scalar engine's HWDGE ring, weights on sync's,
        # so the two rings load in parallel.  First-needed first: w0 and
        # xt[0] per channel tile so the first matmul fires ~9us in.
        wqk_sb = pool_c.tile([128, 8, NC_, 128], BF16)
        wqk_ap = wqk_d.ap()
        xT = pool_x.tile([128, TCH, NC_, 512], BF16)
        xt_ap = xt_d.ap()
        for ci in range(NC_):
            nc.scalar.dma_start(xT[:, 0, ci], xt_ap[0][:, ci])
        for ci in range(NC_):
            nc.sync.dma_start(wqk_sb[:, 0, ci], wqk_ap[0][:, ci])
        nc.sync.dma_start(wqk_sb[:, 4], wqk_ap[4])
        bqk = pool_c.tile([128, 8], F32)
        nc.sync.dma_start(bqk[:], bqk_d.ap()[:])
        bv_sb = pool_c.tile([128, CS], F32)
        nc.sync.dma_start(bv_sb[:], bv_d.ap()[:])
        wv_sb = pool_c.tile([128, NC_, CS], BF16)
        nc.sync.dma_start(wv_sb[:], wv_d.ap()[:])
        mask = pool_c.tile([128, 128], BF16)
        nc.sync.dma_start(mask[:], mk_d.ap()[:])
        for tch in range(1, TCH):
            nc.scalar.dma_start(xT[:, tch], xt_ap[tch])
        for m in (1, 5, 2, 6, 3, 7):
            nc.sync.dma_start(wqk_sb[:, m], wqk_ap[m])
        wp_sb = pool_c.tile([128, 4, C], BF16)
        nc.sync.dma_start(wp_sb[:], wp_d.ap()[:])

        # qkT[p, m, t]: feature-major q|k activations, feature f = m*128+p
        qkT = pool_qk.tile([128, 8, T], BF16)
        yT = pool_y.tile([128, 4, T], BF16)

        # ---------------- QKV projection: one (m-tile, token-chunk) ----------
        def qkv_unit(m, tch):
            ps = ps_sm.tile([128, 512], F32, tag="sm", name=f"qk{m}t{tch}")
            for ci in range(NC_):
                nc.tensor.matmul(
                    ps[:], wqk_sb[:, m, ci], xT[:, tch, ci],
                    start=(ci == 0), stop=(ci == NC_ - 1),
                )
            nc.vector.tensor_scalar(
                out=qkT[:, m, ts(tch, 512)], in0=ps[:],
                scalar1=bqk[:, m : m + 1], scalar2=None, op0=ADD,
            )

        # ---------------- V projection: 4 token blocks of one chunk ----------
        def v_unit(tch):
            for tb_l in range(4):
                tb = tch * 4 + tb_l
                ps = ps_sm.tile([128, 512], F32, tag="sm", name=f"v{tb}")
                for ci in range(NC_):
                    nc.tensor.matmul(
                        ps[:], xT[:, tch, ci, ts(tb_l, 128)], wv_sb[:, ci],
                        start=(ci == 0), stop=(ci == NC_ - 1),
                    )
                nc.vector.tensor_tensor(
                    vext[:, tb].rearrange("p (h s) -> p h s", s=65)[:, :, 0:64],
                    ps[:].rearrange("p (h d) -> p h d", d=64),
                    bv_sb[:].rearrange("p (h d) -> p h d", d=64),
                    ADD,
                )

        # ---------------- attention: one (head pair, query chunk) ------------
        def attn_chunk(p, I):
            nj = 4 * I + 4
            psy = [
                ps_y.tile([65, 512], F32, tag="psy", name=f"psy{p}_{hb_}")
                for hb_ in range(2)
            ]
            # One pss tile per kv block holds BOTH heads ([h0|h1]).  The two
            # QK matmuls then share every dependency, become ready together,
            # and get scheduled adjacently -> their disjoint row groups
            # (0-63 / 64-127, contraction is 64) execute concurrently.
            for j in range(nj):
                r = j - 4 * I
                q0 = max(0, r * 128)
                w = 512 - q0
                pss = ps_qk.tile([128, 1024], F32, tag="qk", name="pss")
                for hb in range(2):
                    base = hb * 64
                    nc.tensor.matmul(
                        pss[:, ds(hb * 512 + q0, w)],
                        qkT[base : base + 64, 4 + p, ts(j, 128)],
                        qkT[base : base + 64, p, ds(I * 512 + q0, w)],
                    )
                P = pool_p.tile([128, 1024], BF16, tag="P", name="P")
                if q0:
                    nc.scalar.activation(
                        P[:].rearrange("k (hb q) -> k hb q", hb=2)[:, :, q0:],
                        pss[:].rearrange("k (hb q) -> k hb q", hb=2)[:, :, q0:],
                        EXP, scale=float(SCALE),
                    )
                else:
                    nc.scalar.activation(P[:], pss[:], EXP, scale=float(SCALE))
                if r >= 0:
                    for hb in range(2):
                        nc.vector.tensor_tensor(
                            P[:, ds(hb * 512 + q0, 128)],
                            P[:, ds(hb * 512 + q0, 128)],
                            mask[:],
                            MULT,
                        )
                for hb in range(2):
                    h = 2 * p + hb
                    nc.tensor.matmul(
                        psy[hb][:, ds(q0, w)],
                        vext[:, j, ds(h * 65, 65)],
                        P[:, ds(hb * 512 + q0, w)],
                        start=(j == 0),
                        stop=(j == nj - 1),
                    )
            # chunk epilogue: evacuate AV psum, batch the two denominator rows
            # into [8,128], one reciprocal, broadcast, normalize, land in yT.
            den2c = pool_r.tile([8, 128], F32R, tag="den2")
            psbs = []
            for hb in range(2):
                psb = pool_tmp.tile([65, 512], F32R, tag="psb", name=f"psb{hb}")
                nc.vector.tensor_copy(out=psb[:], in_=psy[hb][:])
                nc.sync.dma_start(den2c[hb * 4 : hb * 4 + 4, :], psb[64:65, :])
                psbs.append(psb)
            rec2c = pool_r.tile([8, 128], F32R, tag="rec2")
            nc.vector.reciprocal(rec2c[:], den2c[:])
            for hb in range(2):
                rec0 = pool_r.tile([1, 512], F32R, tag="rec0")
                nc.sync.dma_start(rec0[:], rec2c[hb * 4 : hb * 4 + 4, :])
                bc = pool_r.tile([64, 512], F32R, tag="bc")
                nc.gpsimd.partition_broadcast(bc[:], rec0[:])
                yn = pool_tmp.tile([64, 512], BF16, tag="yn")
                nc.vector.tensor_tensor(yn[:], psbs[hb][0:64, :], bc[:], MULT)
                nc.sync.dma_start(
                    yT[hb * 64 : hb * 64 + 64, p, ds(I * 512, 512)], yn[:]
                )

        # ---------------- output projection: all co for one token chunk ------
        outT_ap = outT_d.ap()

        def proj_unit(tch):
            for co in range(C // 128):
                ps = ps_sm.tile([128, 512], F32, tag="sm", name=f"pj{co}t{tch}")
                for cit in range(4):
                    nc.tensor.matmul(
                        ps[:],
                        wp_sb[:, cit, ts(co, 128)],
                        yT[:, cit, ts(tch, 512)],
                        start=(cit == 0),
                        stop=(cit == 3),
                    )
                ot = pool_o.tile([128, 512], BF16, tag="out")
                nc.vector.tensor_copy(out=ot[:], in_=ps[:])
                nc.sync.dma_start(outT_ap[co][:, ts(tch, 512)], ot[:])

        # ---------------- emission = static schedule -------------------------
        # pair 0 interleaved with its own QKV/V feed; remaining m-tiles fill
        # PE gaps in pairs 1-2 (attention alone is ACT-bound); projection
        # chunks follow pair 3's chunks.
        # RAW deps are ordered by emission: every qkv_unit(m, tch) must be
        # emitted BEFORE the first attn_chunk that reads it (Q(p,I) and
        # K(4+p, tch<=I) before attn(p, I)).
        for I in range(TCH):
            qkv_unit(0, I)
            qkv_unit(4, I)
            v_unit(I)
            attn_chunk(0, I)
        for I in range(TCH):
            qkv_unit(1, I)
            qkv_unit(5, I)
            qkv_unit(2, I)
            attn_chunk(1, I)
        for I in range(TCH):
            qkv_unit(6, I)
            qkv_unit(3, I)
            qkv_unit(7, I)
            attn_chunk(2, I)
        # pair-3 chunks run DESCENDING with proj staggered one slot behind,
        # so each proj(I)'s dependency (chunk I's softmax epilogue, a ~5us
        # DVE/DMA latency chain) resolves under the preceding unit's compute
        # and the PE never idles long enough for HAM to re-throttle.
        attn_chunk(3, 3)
        attn_chunk(3, 2)
        proj_unit(3)
        attn_chunk(3, 1)
        proj_unit(2)
        attn_chunk(3, 0)
        proj_unit(1)
        proj_unit(0)

    nc.compile()
    return nc


def kernel(x, w_qkv, b_qkv, w_proj, b_proj):
    x = np.asarray(x, dtype=np.float32)
    w_qkv = np.asarray(w_qkv, dtype=np.float32)
    b_qkv = np.asarray(b_qkv, dtype=np.float32)
    w_proj = np.asarray(w_proj, dtype=np.float32)
    b_proj = np.asarray(b_proj, dtype=np.float32)

    if "nc" not in _CACHE:
        _CACHE["nc"] = _build_program()
    nc = _CACHE["nc"]

    bf = ml_dtypes.bfloat16
    p_i = np.arange(128)[:, None]
    i_i = np.arange(128)[None, :]
    mask = (i_i >= p_i).astype(np.float32).astype(bf)

    in_maps = []
    for c in range(8):
        b, hg = c // 2, c % 2
        sl = slice(hg * CS, (hg + 1) * CS)
        # [Wq | Wk] columns for this head group -> [m, p, ci, f]
        wq = np.concatenate(
            [w_qkv[:, sl], w_qkv[:, C + hg * CS : C + (hg + 1) * CS]], axis=1
        )
        bq = np.concatenate([b_qkv[sl], b_qkv[C + hg * CS : C + (hg + 1) * CS]])
        wv = w_qkv[:, 2 * C + hg * CS : 2 * C + (hg + 1) * CS]
        bv = b_qkv[2 * C + hg * CS : 2 * C + (hg + 1) * CS]
        xt = np.ascontiguousarray(
            x[b].T.reshape(NC_, 128, TCH, 512).transpose(2, 1, 0, 3)
        ).astype(bf)
        wqk_a = np.ascontiguousarray(
            wq.reshape(NC_, 128, 8, 128).transpose(2, 1, 0, 3)
        ).astype(bf)
        bqk_a = np.ascontiguousarray(bq.reshape(8, 128).T).astype(np.float32)
        wv_a = np.ascontiguousarray(
            wv.reshape(NC_, 128, CS).transpose(1, 0, 2)
        ).astype(bf)
        bv_a = np.ascontiguousarray(np.broadcast_to(bv[None, :], (128, CS))).astype(
            np.float32
        )
        wp_a = np.ascontiguousarray(
            w_proj[hg * CS : (hg + 1) * CS].reshape(4, 128, C).transpose(1, 0, 2)
        ).astype(bf)
        in_maps.append({
            "xt": xt,
            "wqk": wqk_a,
            "bqk": bqk_a,
            "wv": wv_a,
            "bv": bv_a,
            "wp": wp_a,
            "mask": mask,
        })

    _CACHE["in_maps"] = in_maps
    res = run_bass_kernel_spmd(nc, in_maps, core_ids=list(range(8)))

    out = np.empty((B, T, C), dtype=np.float32)
    for b in range(B):
        o0 = res.results[2 * b]["outT"].astype(np.float32)
        o1 = res.results[2 * b + 1]["outT"].astype(np.float32)
        # outT[co, p, t] -> [t, co*128+p]
        out[b] = (o0 + o1).transpose(2, 0, 1).reshape(T, C)
        out[b] += b_proj
    return out


# revision 14
# speedup vs baseline: 1.0145x; 1.0145x over previous
"""Causal self-attention (B=4, T=2048, C=1024, H=16, D=64) on 8 TRN2 NeuronCores.

Sharding: data-parallel over batch (4) x tensor-parallel over heads (2 groups
of 8 heads).  Core c handles batch c//2 and heads (c%2)*8 .. (c%2)*8+8.
Each core computes its QKV projection shard, causal flash-style attention for
its 8 heads, and a partial output projection (row-parallel).  The host sums
the two partials per batch and adds b_proj.

v2 over the baseline:
  - QK matmuls for the two heads of a pair are emitted adjacently at row
    groups (0,0)/(64,0) so they execute concurrently (contraction is 64).
  - Causal-diagonal trimming: for query chunk I, the 4 diagonal kv blocks
    stream only the queries that can attend (N = 512-128r), in QK, exp, and
    AV.  Masks shrink to one [128,128] triangle per diagonal block.
  - QKV bias folded into the PSUM->SBUF copy (DVE tensor_scalar); V bias via
    a pre-broadcast [128,512] tile (DVE tensor_tensor).  No bias matmuls.
  - All weights/activations DMA'd from host-prearranged partition-major
    layouts (>=2KB contiguous per partition line); xT streamed per 512-token
    chunk so matmuls start ~4us in.
  - Softmax epilogue per query-chunk (not per pair): yT chunks land early,
    letting the output projection for token chunk tch start right after
    pair 3's chunk tch instead of after the whole pair.
  - Emission interleaves QKV/V units with pair-0 attention chunks, remaining
    QKV m-tiles with pairs 1-2, projection with pair 3 (the per-engine
    queues are static FIFO, so emission order is the schedule).
  - outT stored bf16 (halves output DMA), summed on host in f32.

All matmuls bf16 with f32 PSUM accumulation; scores exp'd on ACT (the only
exp engine); copies pinned to DVE so ACT does exp exclusively.
"""

import os
import sys

for _p in (
    "/root/.axon_site",
    "/root/.axon_site/_ro/trn_rl_repo",
    "/root/.axon_site/_ro/pypackages",
    "/opt/trn_rl_repo",
):
    if os.path.isdir(_p) and _p not in sys.path:
        sys.path.append(_p)

from contextlib import ExitStack

import ml_dtypes
import numpy as np

import concourse.tile as tile
from concourse import bacc, mybir
from concourse.bass import ds, ts
from concourse.bass_utils import run_bass_kernel_spmd

F32 = mybir.dt.float32
F32R = mybir.dt.float32r
BF16 = mybir.dt.bfloat16
EXP = mybir.ActivationFunctionType.Exp
MULT = mybir.AluOpType.mult
ADD = mybir.AluOpType.add

B, T, C, H, D = 4, 2048, 1024, 16, 64
HPC = 8            # heads per core
CS = HPC * D       # 512 sharded channels
NC_ = C // 128     # 8 channel tiles
TB = T // 128      # 16 token blocks
TCH = T // 512     # 4 query chunks
SCALE = 1.0 / np.sqrt(D)

_CACHE = {}


def _build_program():
    nc = bacc.Bacc("TRN2", target_bir_lowering=False, debug=False)

    xt_d = nc.dram_tensor("xt", [TCH, 128, NC_, 512], BF16, kind="ExternalInput")
    wqk_d = nc.dram_tensor("wqk", [8, 128, NC_, 128], BF16, kind="ExternalInput")
    bqk_d = nc.dram_tensor("bqk", [128, 8], F32, kind="ExternalInput")
    wv_d = nc.dram_tensor("wv", [128, NC_, CS], BF16, kind="ExternalInput")
    bv_d = nc.dram_tensor("bv", [128, CS], F32, kind="ExternalInput")
    wp_d = nc.dram_tensor("wp", [128, 4, C], BF16, kind="ExternalInput")
    mk_d = nc.dram_tensor("mask", [128, 128], BF16, kind="ExternalInput")
    outT_d = nc.dram_tensor("outT", [8, 128, T], BF16, kind="ExternalOutput")

    with tile.TileContext(nc) as tc, ExitStack() as ctx, \
            nc.allow_low_precision(reason="bf16 matmuls, f32r epilogue"):
        pool_c = ctx.enter_context(tc.tile_pool(name="const", bufs=1))
        pool_qk = ctx.enter_context(tc.tile_pool(name="qkT", bufs=1))
        pool_ve = ctx.enter_context(tc.tile_pool(name="vext", bufs=1))
        pool_x = ctx.enter_context(tc.tile_pool(name="xT", bufs=1))
        pool_p = ctx.enter_context(tc.tile_pool(name="P", bufs=6))
        pool_r = ctx.enter_context(tc.tile_pool(name="rec", bufs=4))
        pool_tmp = ctx.enter_context(tc.tile_pool(name="psb", bufs=5))
        pool_y = ctx.enter_context(tc.tile_pool(name="yT", bufs=1))
        pool_o = ctx.enter_context(tc.tile_pool(name="out", bufs=3))
        ps_qk = ctx.enter_context(tc.tile_pool(name="psqk", bufs=2, space="PSUM"))
        ps_y = ctx.enter_context(tc.tile_pool(name="psy", bufs=2, space="PSUM"))
        ps_sm = ctx.enter_context(tc.tile_pool(name="pssm", bufs=2, space="PSUM"))

        # v_ext[p, tb, h*65+d]; col h*65+64 holds ones (denominator trick),
        # set by a DVE memset (a DMA here would be descriptor-bound: 16B
        # descriptors that clog the HWDGE ring for ~20us).
        vext = pool_ve.tile([128, TB, HPC * 65], BF16)
        nc.vector.memset(
            vext[:].rearrange("p tb (h s) -> p tb h s", s=65)[:, :, :, 64], 1.0
        )

        # xt streams on the scalar engine's HWDGE ring, weights on sync's,
        # so the two rings load in parallel.  First-needed first: w0 and
        # xt[0] per channel tile so the first matmul fires ~9us in.
        wqk_sb = pool_c.tile([128, 8, NC_, 128], BF16)
        wqk_ap = wqk_d.ap()
        xT = pool_x.tile([128, TCH, NC_, 512], BF16)
        xt_ap = xt_d.ap()
        for ci in range(NC_):
            nc.scalar.dma_start(xT[:, 0, ci], xt_ap[0][:, ci])
        for ci in range(NC_):
            nc.sync.dma_start(wqk_sb[:, 0, ci], wqk_ap[0][:, ci])
        nc.sync.dma_start(wqk_sb[:, 4], wqk_ap[4])
        bqk = pool_c.tile([128, 8], F32)
        nc.sync.dma_start(bqk[:], bqk_d.ap()[:])
        bv_sb = pool_c.tile([128, CS], F32)
        nc.sync.dma_start(bv_sb[:], bv_d.ap()[:])
        mask = pool_c.tile([128, 128], BF16)
        nc.sync.dma_start(mask[:], mk_d.ap()[:])
        # wv rides the scalar ring behind xt[0] so v_unit(0) isn't starved
        # behind the weight queue
        wv_sb = pool_c.tile([128, NC_, CS], BF16)
        nc.scalar.dma_start(wv_sb[:], wv_d.ap()[:])
        for tch in range(1, TCH):
            nc.scalar.dma_start(xT[:, tch], xt_ap[tch])
        for m in (1, 5, 2, 6, 3, 7):
            nc.sync.dma_start(wqk_sb[:, m], wqk_ap[m])
        wp_sb = pool_c.tile([128, 4, C], BF16)
        nc.sync.dma_start(wp_sb[:], wp_d.ap()[:])

        # qkT[p, m, t]: feature-major q|k activations, feature f = m*128+p
        qkT = pool_qk.tile([128, 8, T], BF16)
        yT = pool_y.tile([128, 4, T], BF16)

        # ---------------- QKV projection: one (m-tile, token-chunk) ----------
        def qkv_unit(m, tch):
            ps = ps_sm.tile([128, 512], F32, tag="sm", name=f"qk{m}t{tch}")
            for ci in range(NC_):
                nc.tensor.matmul(
                    ps[:], wqk_sb[:, m, ci], xT[:, tch, ci],
                    start=(ci == 0), stop=(ci == NC_ - 1),
                )
            nc.vector.tensor_scalar(
                out=qkT[:, m, ts(tch, 512)], in0=ps[:],
                scalar1=bqk[:, m : m + 1], scalar2=None, op0=ADD,
            )

        # ---------------- V projection: 4 token blocks of one chunk ----------
        def v_unit(tch):
            for tb_l in range(4):
                tb = tch * 4 + tb_l
                ps = ps_sm.tile([128, 512], F32, tag="sm", name=f"v{tb}")
                for ci in range(NC_):
                    nc.tensor.matmul(
                        ps[:], xT[:, tch, ci, ts(tb_l, 128)], wv_sb[:, ci],
                        start=(ci == 0), stop=(ci == NC_ - 1),
                    )
                nc.vector.tensor_tensor(
                    vext[:, tb].rearrange("p (h s) -> p h s", s=65)[:, :, 0:64],
                    ps[:].rearrange("p (h d) -> p h d", d=64),
                    bv_sb[:].rearrange("p (h d) -> p h d", d=64),
                    ADD,
                )

        # ---------------- attention: one (head pair, query chunk) ------------
        def attn_chunk(p, I):
            nj = 4 * I + 4
            psy = [
                ps_y.tile([65, 512], F32, tag="psy", name=f"psy{p}_{hb_}")
                for hb_ in range(2)
            ]
            # One pss tile per kv block holds BOTH heads ([h0|h1]).  The two
            # QK matmuls then share every dependency, become ready together,
            # and get scheduled adjacently -> their disjoint row groups
            # (0-63 / 64-127, contraction is 64) execute concurrently.
            for j in range(nj):
                r = j - 4 * I
                q0 = max(0, r * 128)
                w = 512 - q0
                pss = ps_qk.tile([128, 1024], F32, tag="qk", name="pss")
                for hb in range(2):
                    base = hb * 64
                    nc.tensor.matmul(
                        pss[:, ds(hb * 512 + q0, w)],
                        qkT[base : base + 64, 4 + p, ts(j, 128)],
                        qkT[base : base + 64, p, ds(I * 512 + q0, w)],
                    )
                P = pool_p.tile([128, 1024], BF16, tag="P", name="P")
                if q0:
                    nc.scalar.activation(
                        P[:].rearrange("k (hb q) -> k hb q", hb=2)[:, :, q0:],
                        pss[:].rearrange("k (hb q) -> k hb q", hb=2)[:, :, q0:],
                        EXP, scale=float(SCALE),
                    )
                else:
                    nc.scalar.activation(P[:], pss[:], EXP, scale=float(SCALE))
                if r >= 0:
                    for hb in range(2):
                        nc.vector.tensor_tensor(
                            P[:, ds(hb * 512 + q0, 128)],
                            P[:, ds(hb * 512 + q0, 128)],
                            mask[:],
                            MULT,
                        )
                for hb in range(2):
                    h = 2 * p + hb
                    nc.tensor.matmul(
                        psy[hb][:, ds(q0, w)],
                        vext[:, j, ds(h * 65, 65)],
                        P[:, ds(hb * 512 + q0, w)],
                        start=(j == 0),
                        stop=(j == nj - 1),
                    )
            # chunk epilogue: evacuate AV psum, batch the two denominator rows
            # into [8,128], one reciprocal, broadcast, normalize, land in yT.
            den2c = pool_r.tile([8, 128], F32R, tag="den2")
            psbs = []
            for hb in range(2):
                psb = pool_tmp.tile([65, 512], F32R, tag="psb", name=f"psb{hb}")
                nc.vector.tensor_copy(out=psb[:], in_=psy[hb][:])
                nc.sync.dma_start(den2c[hb * 4 : hb * 4 + 4, :], psb[64:65, :])
                psbs.append(psb)
            rec2c = pool_r.tile([8, 128], F32R, tag="rec2")
            nc.vector.reciprocal(rec2c[:], den2c[:])
            for hb in range(2):
                rec0 = pool_r.tile([1, 512], F32R, tag="rec0")
                nc.sync.dma_start(rec0[:], rec2c[hb * 4 : hb * 4 + 4, :])
                bc = pool_r.tile([64, 512], F32R, tag="bc")
                nc.gpsimd.partition_broadcast(bc[:], rec0[:])
                yn = pool_tmp.tile([64, 512], BF16, tag="yn")
                nc.vector.tensor_tensor(yn[:], psbs[hb][0:64, :], bc[:], MULT)
                nc.sync.dma_start(
                    yT[hb * 64 : hb * 64 + 64, p, ds(I * 512, 512)], yn[:]
                )

        # ---------------- output projection: all co for one token chunk ------
        outT_ap = outT_d.ap()

        def proj_unit(tch):
            for co in range(C // 128):
                ps = ps_sm.tile([128, 512], F32, tag="sm", name=f"pj{co}t{tch}")
                for cit in range(4):
                    nc.tensor.matmul(
                        ps[:],
                        wp_sb[:, cit, ts(co, 128)],
                        yT[:, cit, ts(tch, 512)],
                        start=(cit == 0),
                        stop=(cit == 3),
                    )
                ot = pool_o.tile([128, 512], BF16, tag="out")
                nc.vector.tensor_copy(out=ot[:], in_=ps[:])
                nc.sync.dma_start(outT_ap[co][:, ts(tch, 512)], ot[:])

        # ---------------- emission = static schedule -------------------------
        # pair 0 interleaved with its own QKV/V feed; remaining m-tiles fill
        # PE gaps in pairs 1-2 (attention alone is ACT-bound); projection
        # chunks follow pair 3's chunks.
        # RAW deps are ordered by emission: every qkv_unit(m, tch) must be
        # emitted BEFORE the first attn_chunk that reads it (Q(p,I) and
        # K(4+p, tch<=I) before attn(p, I)).
        for I in range(TCH):
            qkv_unit(0, I)
            qkv_unit(4, I)
            v_unit(I)
            attn_chunk(0, I)
        for I in range(TCH):
            qkv_unit(1, I)
            qkv_unit(5, I)
            qkv_unit(2, I)
            attn_chunk(1, I)
        for I in range(TCH):
            qkv_unit(6, I)
            qkv_unit(3, I)
            qkv_unit(7, I)
            attn_chunk(2, I)
        # pair-3 chunks run DESCENDING with proj staggered one slot behind,
        # so each proj(I)'s dependency (chunk I's softmax epilogue, a ~5us
        # DVE/DMA latency chain) resolves under the preceding unit's compute
        # and the PE never idles long enough for HAM to re-throttle.
        attn_chunk(3, 3)
        attn_chunk(3, 2)
        proj_unit(3)
        attn_chunk(3, 1)
        proj_unit(2)
        attn_chunk(3, 0)
        proj_unit(1)
        proj_unit(0)

    nc.compile()
    return nc


def kernel(x, w_qkv, b_qkv, w_proj, b_proj):
    x = np.asarray(x, dtype=np.float32)
    w_qkv = np.asarray(w_qkv, dtype=np.float32)
    b_qkv = np.asarray(b_qkv, dtype=np.float32)
    w_proj = np.asarray(w_proj, dtype=np.float32)
    b_proj = np.asarray(b_proj, dtype=np.float32)

    if "nc" not in _CACHE:
        _CACHE["nc"] = _build_program()
    nc = _CACHE["nc"]

    bf = ml_dtypes.bfloat16
    p_i = np.arange(128)[:, None]
    i_i = np.arange(128)[None, :]
    mask = (i_i >= p_i).astype(np.float32).astype(bf)

    in_maps = []
    for c in range(8):
        b, hg = c // 2, c % 2
        sl = slice(hg * CS, (hg + 1) * CS)
        # [Wq | Wk] columns for this head group -> [m, p, ci, f]
        wq = np.concatenate(
            [w_qkv[:, sl], w_qkv[:, C + hg * CS : C + (hg + 1) * CS]], axis=1
        )
        bq = np.concatenate([b_qkv[sl], b_qkv[C + hg * CS : C + (hg + 1) * CS]])
        wv = w_qkv[:, 2 * C + hg * CS : 2 * C + (hg + 1) * CS]
        bv = b_qkv[2 * C + hg * CS : 2 * C + (hg + 1) * CS]
        xt = np.ascontiguousarray(
            x[b].T.reshape(NC_, 128, TCH, 512).transpose(2, 1, 0, 3)
        ).astype(bf)
        wqk_a = np.ascontiguousarray(
            wq.reshape(NC_, 128, 8, 128).transpose(2, 1, 0, 3)
        ).astype(bf)
        bqk_a = np.ascontiguousarray(bq.reshape(8, 128).T).astype(np.float32)
        wv_a = np.ascontiguousarray(
            wv.reshape(NC_, 128, CS).transpose(1, 0, 2)
        ).astype(bf)
        bv_a = np.ascontiguousarray(np.broadcast_to(bv[None, :], (128, CS))).astype(
            np.float32
        )
        wp_a = np.ascontiguousarray(
            w_proj[hg * CS : (hg + 1) * CS].reshape(4, 128, C).transpose(1, 0, 2)
        ).astype(bf)
        in_maps.append({
            "xt": xt,
            "wqk": wqk_a,
            "bqk": bqk_a,
            "wv": wv_a,
            "bv": bv_a,
            "wp": wp_a,
            "mask": mask,
        })

    _CACHE["in_maps"] = in_maps
    res = run_bass_kernel_spmd(nc, in_maps, core_ids=list(range(8)))

    out = np.empty((B, T, C), dtype=np.float32)
    for b in range(B):
        o0 = res.results[2 * b]["outT"].astype(np.float32)
        o1 = res.results[2 * b + 1]["outT"].astype(np.float32)
        # outT[co, p, t] -> [t, co*128+p]
        out[b] = (o0 + o1).transpose(2, 0, 1).reshape(T, C)
        out[b] += b_proj
    return out
